# revision 22
# baseline (speedup 1.0000x reference)
"""Batched Chamfer loss on 8 Trainium2 cores.

Strategy (data-parallel over batch, 2 batches/core):
  d2[n,m] = ||s_n||^2 + ||t_m||^2 - 2 s_n.t_m is produced directly by one
  K=18 bf16 matmul per [128,512] tile using a split-precision packing
  (exact products, fp32 accum).  ScalarE evacuates PSUM -> SBUF as bf16
  *negated* (scale=-1), so both chamfer directions become max-reductions.

  Per [128,4096] span tile, VectorE does:
    - running col-max: one full-width tensor_tensor max (bf16 2x mode);
      the first tile uses tensor_copy instead of memset+max.
    - row-max: a tt-max pyramid (4096->2048->1024, 2x mode); each tile's
      1024-wide tail lands in a per-batch rowcat buffer [128, 32*1024].
      After the tile loop, strided 3D-view in-place tensor_tensor ops
      (which keep 2x mode) fold every tile's chunk 1024->32, then one
      small 1x tensor_reduce produces rowm.  This batched 2x fold beats
      per-tile finishers: any DVE op with a horizontal-reduce output runs
      1x; only plain 2-src tensor_tensor on bf16 gets 2x packed mode.

  All dtypes are bfloat16: IEEE float16 hits pathologically slow DVE paths
  on this hardware.  tensor_tensor_reduce (which would fuse pyramid+accum)
  hangs the DVE here - do not use it.

  GPSIMD folds the per-batch col-max across partitions
  (partition_all_reduce(max)); the fold of batch b overlaps batch b+1.
  Host negates and does the tiny final means.

  Engine budget per core/rep (measured): DVE ~272us (bottleneck, ~2.07
  passes over the 33.5M distances at 2 elem/lane/cycle), ACT evac ~205us,
  PE matmuls ~110us, GPSIMD folds ~27us.
"""
import numpy as np
import ml_dtypes

B, N, M = 16, 4096, 4096
NCORES = 8
BPC = B // NCORES          # batches per core
K = 18                     # packed contraction rows
NT = N // 128              # 32 n-tiles
NH = 2                     # m halves
HW = M // NH               # 2048 columns per half
BF16 = ml_dtypes.bfloat16

_cache = {}


def _split2(x):
    """fp32 array -> (hi, lo) bf16 so hi+lo ~ x to ~2^-17 rel."""
    hi = x.astype(BF16)
    lo = (x - hi.astype(np.float32)).astype(BF16)
    return hi, lo


def _split3(x):
    a = x.astype(BF16)
    r = x - a.astype(np.float32)
    b = r.astype(BF16)
    c = (r - b.astype(np.float32)).astype(BF16)
    return a, b, c


def _pack_all(src, tgt):
    """src: [B,N,3] f32, tgt: [B,M,3] f32 -> apack [B,K,N], bpack [B,K,M] bf16."""
    a = np.zeros((B, K, N), dtype=BF16)
    bp = np.zeros((B, K, M), dtype=BF16)
    for d in range(3):
        xh, xl = _split2(-2.0 * src[:, :, d])
        th, tl = _split2(tgt[:, :, d])
        r = 4 * d
        a[:, r + 0] = xh
        a[:, r + 1] = xh
        a[:, r + 2] = xl
        a[:, r + 3] = xl
        bp[:, r + 0] = th
        bp[:, r + 1] = tl
        bp[:, r + 2] = th
        bp[:, r + 3] = tl
    s2 = np.einsum("bnd,bnd->bn", src, src, dtype=np.float64).astype(np.float32)
    t2 = np.einsum("bmd,bmd->bm", tgt, tgt, dtype=np.float64).astype(np.float32)
    s2a, s2b, s2c = _split3(s2)
    a[:, 12], a[:, 13], a[:, 14] = s2a, s2b, s2c
    bp[:, 12:15] = np.ones((B, 3, M), dtype=BF16)
    a[:, 15:18] = np.ones((B, 3, N), dtype=BF16)
    t2a, t2b, t2c = _split3(t2)
    bp[:, 15], bp[:, 16], bp[:, 17] = t2a, t2b, t2c
    return a, bp


def _build(reps=1, pyr_stop=1024, span_bufs=4, colmax_half=False, tred=True, fold_host=False, scr_bufs=2, cm_bufs=2, gp_tiles=0):
    import concourse.bacc as bacc
    import concourse.mybir as mybir
    import concourse.tile as tile
    import concourse.bass_isa as bass_isa

    assert not tred or pyr_stop in (64, 128, 256, 512, 1024), "tred needs pyr_stop<=1024"
    f32 = mybir.dt.float32
    bf16 = mybir.dt.bfloat16
    MAX = mybir.AluOpType.max
    BYP = mybir.AluOpType.bypass

    nc = bacc.Bacc("TRN2", target_bir_lowering=False, debug=False)
    apack = nc.dram_tensor("apack", [BPC, K, N], bf16, kind="ExternalInput")
    bpack = nc.dram_tensor("bpack", [BPC, K, M], bf16, kind="ExternalInput")
    # outputs hold NEGATED mins
    o_rm = nc.dram_tensor("rowmins", [BPC, 128, NT], f32, kind="ExternalOutput")
    cm_rows = 128 if fold_host else 1 + gp_tiles
    o_cm = nc.dram_tensor("colmin", [BPC, cm_rows, M] if cm_rows > 1 else [BPC, M],
                          bf16, kind="ExternalOutput")

    with tile.TileContext(nc) as tc:
        with (
            tc.tile_pool(name="w", bufs=2) as wpool,
            tc.tile_pool(name="ps", bufs=2, space="PSUM") as pspool,
            tc.tile_pool(name="span", bufs=span_bufs) as sppool,
            tc.tile_pool(name="scr", bufs=scr_bufs) as scrpool,
            tc.tile_pool(name="coll", bufs=2) as clpool,
            tc.tile_pool(name="cm", bufs=cm_bufs) as cmpool,
            tc.tile_pool(name="rm", bufs=2) as rmpool,
            tc.tile_pool(name="rc", bufs=1 if pyr_stop >= 512 else 2) as rcpool,
            tc.tile_pool(name="fold", bufs=2) as fpool,
        ):
            for b_rep in range(BPC * reps):
                b = b_rep % BPC
                a_sb = wpool.tile([K, N], bf16, tag="a", name="a_sb")
                bt_sb = wpool.tile([K, M], bf16, tag="bt", name="bt_sb")
                nc.sync.dma_start(a_sb[:], apack[b])
                nc.sync.dma_start(bt_sb[:], bpack[b])

                gp_set = set(range(2, NT, NT // gp_tiles)[:gp_tiles] if gp_tiles else [])
                if gp_tiles:
                    gp_list = sorted(gp_set)
                    coll = clpool.tile([gp_tiles, M], bf16, tag="coll", name="coll")
                colmax = cmpool.tile([128, M], bf16, tag="cm", name="colmax")
                rowm = rmpool.tile([128, NT], f32, tag="rm", name="rowm")
                rowcat = (rcpool.tile([128, NT * pyr_stop], bf16, tag="rc", name="rowcat")
                          if tred else None)

                for t in range(NT):
                    span = sppool.tile([128, M], bf16, tag="span", name="span")
                    for h in range(NH):
                        ps = pspool.tile([128, HW], f32, tag="ps", name="ps")
                        for i in range(HW // 512):
                            nc.tensor.matmul(
                                ps[:, i * 512:(i + 1) * 512],
                                a_sb[:, t * 128:(t + 1) * 128],
                                bt_sb[:, h * HW + i * 512: h * HW + (i + 1) * 512],
                                start=True,
                                stop=True,
                            )
                        # evacuate + negate: span = -d2
                        nc.scalar.mul(span[:, h * HW:(h + 1) * HW], ps[:], -1.0)
                        # running col-max for this half
                        if colmax_half:
                            if t == 0:
                                nc.vector.tensor_copy(
                                    colmax[:, h * HW:(h + 1) * HW],
                                    span[:, h * HW:(h + 1) * HW],
                                )
                            else:
                                nc.vector.tensor_tensor(
                                    out=colmax[:, h * HW:(h + 1) * HW],
                                    in0=span[:, h * HW:(h + 1) * HW],
                                    in1=colmax[:, h * HW:(h + 1) * HW],
                                    op=MAX,
                                )
                    if not colmax_half:
                        if t == 0:
                            nc.vector.tensor_copy(colmax[:], span[:])
                        elif gp_tiles and t in gp_set:
                            cft = fpool.tile([128, M], bf16, tag="cft", name="cft")
                            nc.gpsimd.partition_all_reduce(
                                cft[:], span[:], 128, bass_isa.ReduceOp.max
                            )
                            j = gp_list.index(t)
                            nc.sync.dma_start(coll[j:j + 1, :], cft[0:1, :])
                        else:
                            nc.vector.tensor_tensor(
                                out=colmax[:], in0=span[:], in1=colmax[:], op=MAX
                            )
                    # row-max pyramid: tt-max halves down to pyr_stop wide,
                    # then one small ts max-accumulate into rowm[:, t].
                    scr = scrpool.tile([128, HW], bf16, tag="scr", name="scr")
                    cur, width, off = span, M, 0   # (tensor, width, offset of level)
                    while width > pyr_stop:
                        half = width // 2
                        last = half == pyr_stop and tred
                        if last:
                            dst = rowcat[:, t * pyr_stop:(t + 1) * pyr_stop]
                        else:
                            dst_off = 0 if cur is span else off + width
                            dst = scr[:, dst_off:dst_off + half]
                        nc.vector.tensor_tensor(
                            out=dst,
                            in0=cur[:, off:off + half],
                            in1=cur[:, off + half:off + width],
                            op=MAX,
                        )
                        if last:
                            break
                        cur, width, off = scr, half, dst_off
                    if not tred:
                        ts_out = (scr[:, off + width:off + 2 * width] if cur is scr
                                  else scr[:, 0:width])
                        nc.vector.tensor_scalar(
                            out=ts_out,
                            in0=cur[:, off:off + width],
                            scalar1=0.0,
                            scalar2=None,
                            op0=BYP,
                            op1=MAX,
                            accum_out=rowm[:, t:t + 1],
                        )
                if tred:
                    # fold the per-tile chunks in place (strided 3D views keep
                    # tt in 2x mode) down to 32 wide, then one small 1x tred
                    v = rowcat[:].rearrange("p (c w) -> p c w", w=pyr_stop)
                    w = pyr_stop
                    while w > 32:
                        h2 = w // 2
                        nc.vector.tensor_tensor(
                            out=v[:, :, 0:h2], in0=v[:, :, 0:h2],
                            in1=v[:, :, h2:w], op=MAX,
                        )
                        w = h2
                    nc.vector.tensor_reduce(
                        out=rowm[:],
                        in_=v[:, :, 0:32],
                        op=MAX,
                        axis=mybir.AxisListType.X,
                    )

                # fold col-max across partitions on gpsimd (or ship raw to host)
                if fold_host:
                    nc.sync.dma_start(o_cm[b], colmax[:])
                else:
                    cfold = fpool.tile([128, M], bf16, tag="cf", name="cfold")
                    nc.gpsimd.partition_all_reduce(
                        cfold[:], colmax[:], 128, bass_isa.ReduceOp.max
                    )
                    if gp_tiles:
                        nc.sync.dma_start(o_cm[b][0:1, :], cfold[0:1, :])
                        nc.sync.dma_start(o_cm[b][1:1 + gp_tiles, :], coll[:])
                    else:
                        nc.sync.dma_start(o_cm[b], cfold[0:1, :])
                nc.sync.dma_start(o_rm[b], rowm[:])
    nc.compile()
    return nc


def _get_module():
    if "nc" not in _cache:
        _cache["nc"] = _build()
    return _cache["nc"]


def _make_in_maps(src_points, tgt_points):
    a, bp = _pack_all(np.asarray(src_points, np.float32), np.asarray(tgt_points, np.float32))
    return [
        {"apack": a[c * BPC:(c + 1) * BPC], "bpack": bp[c * BPC:(c + 1) * BPC]}
        for c in range(NCORES)
    ]


def _host_reduce(results, weights):
    total = 0.0
    for c in range(NCORES):
        rm = results[c]["rowmins"].astype(np.float64)  # [BPC,128,NT], -rowmin
        cm = results[c]["colmin"].astype(np.float64)   # [BPC,M] or [BPC,128,M], -colmin
        if cm.ndim == 3:
            cm = cm.max(axis=1)
        for j in range(BPC):
            b = c * BPC + j
            d1 = np.maximum(-rm[j], 0.0).mean()
            d2 = np.maximum(-cm[j], 0.0).mean()
            total += float(weights[b]) * (d1 + d2)
    return np.float32(total / B)


def kernel(src_points, tgt_points, weights):
    from concourse.bass_utils import run_bass_kernel_spmd

    src_points = np.asarray(src_points, dtype=np.float32)
    tgt_points = np.asarray(tgt_points, dtype=np.float32)
    weights = np.asarray(weights, dtype=np.float32)

    nc = _get_module()
    in_maps = _make_in_maps(src_points, tgt_points)
    res = run_bass_kernel_spmd(nc, in_maps, list(range(NCORES)))
    return _host_reduce(res.results, weights)


# revision 23
# speedup vs baseline: 1.0283x; 1.0283x over previous
"""Batched Chamfer loss on 8 Trainium2 cores.

Strategy (data-parallel over batch, 2 batches/core):
  d2[n,m] = ||s_n||^2 + ||t_m||^2 - 2 s_n.t_m is produced directly by one
  K=18 bf16 matmul per [128,512] tile using a split-precision packing
  (exact products, fp32 accum).  ScalarE evacuates PSUM -> SBUF as bf16
  *negated* (scale=-1), so both chamfer directions become max-reductions.

  Per [128,4096] span tile, VectorE does:
    - running col-max: one full-width tensor_tensor max (bf16 2x mode);
      the first tile uses tensor_copy instead of memset+max.
    - row-max: a tt-max pyramid (4096->2048->1024, 2x mode); each tile's
      1024-wide tail lands in a per-batch rowcat buffer [128, 32*1024].
      After the tile loop, strided 3D-view in-place tensor_tensor ops
      (which keep 2x mode) fold every tile's chunk 1024->32, then one
      small 1x tensor_reduce produces rowm.  This batched 2x fold beats
      per-tile finishers: any DVE op with a horizontal-reduce output runs
      1x; only plain 2-src tensor_tensor on bf16 gets 2x packed mode.

  All dtypes are bfloat16: IEEE float16 hits pathologically slow DVE paths
  on this hardware.  tensor_tensor_reduce (which would fuse pyramid+accum)
  hangs the DVE here - do not use it.

  GPSIMD folds the per-batch col-max across partitions
  (partition_all_reduce(max)); the fold of batch b overlaps batch b+1.
  Host negates and does the tiny final means.

  Engine budget per core/rep (measured): DVE ~272us (bottleneck, ~2.07
  passes over the 33.5M distances at 2 elem/lane/cycle), ACT evac ~205us,
  PE matmuls ~110us, GPSIMD folds ~27us.
"""
import numpy as np
import ml_dtypes

B, N, M = 16, 4096, 4096
NCORES = 8
BPC = B // NCORES          # batches per core
K = 18                     # packed contraction rows
NT = N // 128              # 32 n-tiles
NH = 2                     # m halves
HW = M // NH               # 2048 columns per half
BF16 = ml_dtypes.bfloat16

_cache = {}


def _split2(x):
    """fp32 array -> (hi, lo) bf16 so hi+lo ~ x to ~2^-17 rel."""
    hi = x.astype(BF16)
    lo = (x - hi.astype(np.float32)).astype(BF16)
    return hi, lo


def _split3(x):
    a = x.astype(BF16)
    r = x - a.astype(np.float32)
    b = r.astype(BF16)
    c = (r - b.astype(np.float32)).astype(BF16)
    return a, b, c


def _pack_all(src, tgt):
    """src: [B,N,3] f32, tgt: [B,M,3] f32 -> apack [B,K,N], bpack [B,K,M] bf16."""
    a = np.zeros((B, K, N), dtype=BF16)
    bp = np.zeros((B, K, M), dtype=BF16)
    for d in range(3):
        xh, xl = _split2(-2.0 * src[:, :, d])
        th, tl = _split2(tgt[:, :, d])
        r = 4 * d
        a[:, r + 0] = xh
        a[:, r + 1] = xh
        a[:, r + 2] = xl
        a[:, r + 3] = xl
        bp[:, r + 0] = th
        bp[:, r + 1] = tl
        bp[:, r + 2] = th
        bp[:, r + 3] = tl
    s2 = np.einsum("bnd,bnd->bn", src, src, dtype=np.float64).astype(np.float32)
    t2 = np.einsum("bmd,bmd->bm", tgt, tgt, dtype=np.float64).astype(np.float32)
    s2a, s2b, s2c = _split3(s2)
    a[:, 12], a[:, 13], a[:, 14] = s2a, s2b, s2c
    bp[:, 12:15] = np.ones((B, 3, M), dtype=BF16)
    a[:, 15:18] = np.ones((B, 3, N), dtype=BF16)
    t2a, t2b, t2c = _split3(t2)
    bp[:, 15], bp[:, 16], bp[:, 17] = t2a, t2b, t2c
    return a, bp


def _build(reps=1, pyr_stop=1024, span_bufs=4, colmax_half=False, tred=True, fold_host=False, scr_bufs=2, cm_bufs=2, gp_tiles=0):
    import concourse.bacc as bacc
    import concourse.mybir as mybir
    import concourse.tile as tile
    import concourse.bass_isa as bass_isa

    assert not tred or pyr_stop in (64, 128, 256, 512, 1024), "tred needs pyr_stop<=1024"
    f32 = mybir.dt.float32
    bf16 = mybir.dt.bfloat16
    MAX = mybir.AluOpType.max
    BYP = mybir.AluOpType.bypass

    nc = bacc.Bacc("TRN2", target_bir_lowering=False, debug=False)
    apack = nc.dram_tensor("apack", [BPC, K, N], bf16, kind="ExternalInput")
    bpack = nc.dram_tensor("bpack", [BPC, K, M], bf16, kind="ExternalInput")
    # outputs hold NEGATED mins
    o_rm = nc.dram_tensor("rowmins", [BPC, 128, NT], f32, kind="ExternalOutput")
    cm_rows = 128 if fold_host else 1 + gp_tiles
    o_cm = nc.dram_tensor("colmin", [BPC, cm_rows, M] if cm_rows > 1 else [BPC, M],
                          bf16, kind="ExternalOutput")

    with tile.TileContext(nc) as tc:
        with (
            tc.tile_pool(name="w", bufs=2) as wpool,
            tc.tile_pool(name="ps", bufs=2, space="PSUM") as pspool,
            tc.tile_pool(name="span", bufs=span_bufs) as sppool,
            tc.tile_pool(name="scr", bufs=scr_bufs) as scrpool,
            tc.tile_pool(name="coll", bufs=2) as clpool,
            tc.tile_pool(name="cm", bufs=cm_bufs) as cmpool,
            tc.tile_pool(name="rm", bufs=2) as rmpool,
            tc.tile_pool(name="rc", bufs=1 if pyr_stop >= 512 else 2) as rcpool,
            tc.tile_pool(name="fold", bufs=2) as fpool,
        ):
            for b_rep in range(BPC * reps):
                b = b_rep % BPC
                a_sb = wpool.tile([K, N], bf16, tag="a", name="a_sb")
                bt_sb = wpool.tile([K, M], bf16, tag="bt", name="bt_sb")
                nc.sync.dma_start(a_sb[:], apack[b])
                nc.sync.dma_start(bt_sb[:], bpack[b])

                gp_set = set(range(2, NT, NT // gp_tiles)[:gp_tiles] if gp_tiles else [])
                if gp_tiles:
                    gp_list = sorted(gp_set)
                    coll = clpool.tile([gp_tiles, M], bf16, tag="coll", name="coll")
                colmax = cmpool.tile([128, M], bf16, tag="cm", name="colmax")
                rowm = rmpool.tile([128, NT], f32, tag="rm", name="rowm")
                rowcat = (rcpool.tile([128, NT * pyr_stop], bf16, tag="rc", name="rowcat")
                          if tred else None)

                for t in range(NT):
                    span = sppool.tile([128, M], bf16, tag="span", name="span")
                    for h in range(NH):
                        ps = pspool.tile([128, HW], f32, tag="ps", name="ps")
                        for i in range(HW // 512):
                            nc.tensor.matmul(
                                ps[:, i * 512:(i + 1) * 512],
                                a_sb[:, t * 128:(t + 1) * 128],
                                bt_sb[:, h * HW + i * 512: h * HW + (i + 1) * 512],
                                start=True,
                                stop=True,
                            )
                        # evacuate + negate: span = -d2
                        nc.scalar.mul(span[:, h * HW:(h + 1) * HW], ps[:], -1.0)
                        # running col-max for this half
                        if colmax_half:
                            if t == 0:
                                nc.vector.tensor_copy(
                                    colmax[:, h * HW:(h + 1) * HW],
                                    span[:, h * HW:(h + 1) * HW],
                                )
                            else:
                                nc.vector.tensor_tensor(
                                    out=colmax[:, h * HW:(h + 1) * HW],
                                    in0=span[:, h * HW:(h + 1) * HW],
                                    in1=colmax[:, h * HW:(h + 1) * HW],
                                    op=MAX,
                                )
                    if not colmax_half:
                        if t == 0:
                            nc.vector.tensor_copy(colmax[:], span[:])
                        elif gp_tiles and t in gp_set:
                            cft = fpool.tile([128, M], bf16, tag="cft", name="cft")
                            nc.gpsimd.partition_all_reduce(
                                cft[:], span[:], 128, bass_isa.ReduceOp.max
                            )
                            j = gp_list.index(t)
                            nc.sync.dma_start(coll[j:j + 1, :], cft[0:1, :])
                        else:
                            nc.vector.tensor_tensor(
                                out=colmax[:], in0=span[:], in1=colmax[:], op=MAX
                            )
                    # row-max pyramid: tt-max halves down to pyr_stop wide,
                    # then one small ts max-accumulate into rowm[:, t].
                    scr_w = HW if (tred and pyr_stop >= 1024) else M
                    scr = scrpool.tile([128, scr_w], bf16, tag="scr", name="scr")
                    cur, width, off = span, M, 0   # (tensor, width, offset of level)
                    while width > pyr_stop:
                        half = width // 2
                        last = half == pyr_stop and tred
                        if last:
                            dst = rowcat[:, t * pyr_stop:(t + 1) * pyr_stop]
                        else:
                            dst_off = 0 if cur is span else off + width
                            dst = scr[:, dst_off:dst_off + half]
                        nc.vector.tensor_tensor(
                            out=dst,
                            in0=cur[:, off:off + half],
                            in1=cur[:, off + half:off + width],
                            op=MAX,
                        )
                        if last:
                            break
                        cur, width, off = scr, half, dst_off
                    if not tred:
                        ts_out = (scr[:, off + width:off + 2 * width] if cur is scr
                                  else scr[:, 0:width])
                        nc.vector.tensor_scalar(
                            out=ts_out,
                            in0=cur[:, off:off + width],
                            scalar1=0.0,
                            scalar2=None,
                            op0=BYP,
                            op1=MAX,
                            accum_out=rowm[:, t:t + 1],
                        )
                if tred:
                    # fold the per-tile chunks in place (strided 3D views keep
                    # tt in 2x mode) down to 32 wide, then one small 1x tred
                    v = rowcat[:].rearrange("p (c w) -> p c w", w=pyr_stop)
                    w = pyr_stop
                    while w > 32:
                        h2 = w // 2
                        nc.vector.tensor_tensor(
                            out=v[:, :, 0:h2], in0=v[:, :, 0:h2],
                            in1=v[:, :, h2:w], op=MAX,
                        )
                        w = h2
                    nc.vector.tensor_reduce(
                        out=rowm[:],
                        in_=v[:, :, 0:32],
                        op=MAX,
                        axis=mybir.AxisListType.X,
                    )

                # fold col-max across partitions on gpsimd (or ship raw to host)
                if fold_host:
                    nc.sync.dma_start(o_cm[b], colmax[:])
                else:
                    cfold = fpool.tile([128, M], bf16, tag="cf", name="cfold")
                    nc.gpsimd.partition_all_reduce(
                        cfold[:], colmax[:], 128, bass_isa.ReduceOp.max
                    )
                    if gp_tiles:
                        nc.sync.dma_start(o_cm[b][0:1, :], cfold[0:1, :])
                        nc.sync.dma_start(o_cm[b][1:1 + gp_tiles, :], coll[:])
                    else:
                        nc.sync.dma_start(o_cm[b], cfold[0:1, :])
                nc.sync.dma_start(o_rm[b], rowm[:])
    nc.compile()
    return nc


def _get_module():
    if "nc" not in _cache:
        _cache["nc"] = _build()
    return _cache["nc"]


def _make_in_maps(src_points, tgt_points):
    a, bp = _pack_all(np.asarray(src_points, np.float32), np.asarray(tgt_points, np.float32))
    return [
        {"apack": a[c * BPC:(c + 1) * BPC], "bpack": bp[c * BPC:(c + 1) * BPC]}
        for c in range(NCORES)
    ]


def _host_reduce(results, weights):
    total = 0.0
    for c in range(NCORES):
        rm = results[c]["rowmins"].astype(np.float64)  # [BPC,128,NT], -rowmin
        cm = results[c]["colmin"].astype(np.float64)   # [BPC,M] or [BPC,128,M], -colmin
        if cm.ndim == 3:
            cm = cm.max(axis=1)
        for j in range(BPC):
            b = c * BPC + j
            d1 = np.maximum(-rm[j], 0.0).mean()
            d2 = np.maximum(-cm[j], 0.0).mean()
            total += float(weights[b]) * (d1 + d2)
    return np.float32(total / B)


def kernel(src_points, tgt_points, weights):
    from concourse.bass_utils import run_bass_kernel_spmd

    src_points = np.asarray(src_points, dtype=np.float32)
    tgt_points = np.asarray(tgt_points, dtype=np.float32)
    weights = np.asarray(weights, dtype=np.float32)

    nc = _get_module()
    in_maps = _make_in_maps(src_points, tgt_points)
    res = run_bass_kernel_spmd(nc, in_maps, list(range(NCORES)))
    return _host_reduce(res.results, weights)
